# revision 2
# baseline (speedup 1.0000x reference)
"""HGNN (2-stage hypergraph conv) kernel for Trainium2 — single-core fp16.

The axon-tunneled dispatch path re-ships every bound DRAM tensor on every
execution (~14 GB/s) and has a per-core serialized call overhead, so the
dominant cost is wire bytes + per-call floor, not device compute (the whole
model is ~4 ms of PE time). Strategy:
  - ONE NeuronCore (floor(1) ~= 4.9 ms vs 8.3 ms for 8 cores; no 8x weight
    replication).
  - Ship x3/x4/weights/biases/output as fp16 (146 MB total vs 582 MB fp32
    8-core). H ships fp32; G is computed on device in fp32 then cast.
  - Device kernel: 8 chunks of 16 batches, same 3-phase structure per chunk:
      A: A_fm[d,(b,m)] = (G X_b)^T   (activation-stationary, RM->FM)
      B: H_fm = relu(A_fm^T W1 + g (x) b1)  (weight-stationary, FM->FM)
      C: Y = H^T W2 + b2; Z = G Y; DMA out  (FM->RM)
    fp16 matmuls run at 1 cycle/row on the PE even for 80-wide frees
    (f32r pays 4x below 256), accumulation stays fp32 in PSUM.
"""
import numpy as np

_CACHE = {}

NN = 80
B_FULL = 128
B_CHUNK = 16
N_CHUNKS = B_FULL // B_CHUNK
R = B_CHUNK * NN  # 1280


def _build_program():
    import concourse.mybir as mybir
    import concourse.tile as tile
    from concourse import bacc
    from concourse.masks import make_identity

    dt = mybir.dt
    AF = mybir.ActivationFunctionType
    ALU = mybir.AluOpType
    f16 = dt.float16
    f32 = dt.float32

    B = B_CHUNK
    RCHUNKS = [(0, 512), (512, 512), (1024, 256)]
    BGROUPS = [(0, 6), (6, 6), (12, 4)]

    nc = bacc.Bacc("TRN2", target_bir_lowering=False, debug=False)

    x3_d = nc.dram_tensor("x3", [B_FULL, NN, 1024], f16, kind="ExternalInput").ap()
    x4_d = nc.dram_tensor("x4", [B_FULL, NN, 2048], f16, kind="ExternalInput").ap()
    H_d = nc.dram_tensor("H", [NN, NN], f32, kind="ExternalInput").ap()
    w31_d = nc.dram_tensor("w31", [1024, 1024], f16, kind="ExternalInput").ap()
    w32_d = nc.dram_tensor("w32", [1024, 1024], f16, kind="ExternalInput").ap()
    w41_d = nc.dram_tensor("w41", [2048, 2048], f16, kind="ExternalInput").ap()
    w42_d = nc.dram_tensor("w42", [2048, 2048], f16, kind="ExternalInput").ap()
    b31_d = nc.dram_tensor("b31", [1, 1024], f16, kind="ExternalInput").ap()
    b32_d = nc.dram_tensor("b32", [1, 1024], f16, kind="ExternalInput").ap()
    b41_d = nc.dram_tensor("b41", [1, 2048], f16, kind="ExternalInput").ap()
    b42_d = nc.dram_tensor("b42", [1, 2048], f16, kind="ExternalInput").ap()
    out_d = nc.dram_tensor("out", [B_FULL, NN, 3072], f16, kind="ExternalOutput").ap()

    with tile.TileContext(nc) as tc:
        with tc.tile_pool(name="const", bufs=1) as cpool:
            G_h = cpool.tile([NN, NN], f16)
            GP_SHIFTS = [0, 16, 32, 48, 64, 80, 96, 112, -16, -32, -48, -64]
            gpad = {}
            for s in GP_SHIFTS:
                gpad[s] = cpool.tile([128, NN], f16, tag=f"gpad{s}", name=f"gpad{s}")
            grow_h = cpool.tile([1, R], f16)
            ones128_h = cpool.tile([1, 128], f16)
            b1s = {}
            b2s = {}
            for D, b1_d, b2_d in ((1024, b31_d, b32_d), (2048, b41_d, b42_d)):
                b1s[D] = cpool.tile([1, D], f16, tag=f"b1{D}", name=f"b1{D}")
                b2s[D] = cpool.tile([1, D], f16, tag=f"b2{D}", name=f"b2{D}")
                nc.sync.dma_start(b1s[D][:], b1_d)
                nc.sync.dma_start(b2s[D][:], b2_d)

            # ---- G setup (tiny, fp32) ----
            with tc.tile_pool(name="gsetup", bufs=1) as gp, \
                 tc.tile_pool(name="gps", bufs=1, space="PSUM") as gpsum:
                ident = gp.tile([NN, NN], f32)
                make_identity(nc, ident[:])
                ones_col = gp.tile([NN, 1], f32)
                nc.vector.memset(ones_col[:], 1.0)
                Hsb = gp.tile([NN, NN], f32)
                nc.sync.dma_start(Hsb[:], H_d)
                Hs = gp.tile([NN, NN], f32)
                nc.scalar.activation(Hs[:], Hsb[:], AF.Sigmoid)
                dv = gp.tile([NN, 1], f32)
                nc.vector.tensor_reduce(dv[:], Hs[:], mybir.AxisListType.X, ALU.add)
                sq = gp.tile([NN, 1], f32)
                nc.scalar.sqrt(sq[:], dv[:])
                dv2 = gp.tile([NN, 1], f32)
                nc.vector.reciprocal(dv2[:], sq[:])
                Hp = gp.tile([NN, NN], f32)
                nc.scalar.mul(Hp[:], Hs[:], dv2[:])  # Hs * dv2[n]
                ps_de = gpsum.tile([NN, 1], f32)
                nc.tensor.matmul(ps_de[:], Hs[:], ones_col[:], start=True, stop=True)
                inv_de = gp.tile([NN, 1], f32)
                nc.vector.reciprocal(inv_de[:], ps_de[:])
                ps_hpt = gpsum.tile([NN, NN], f32)
                nc.tensor.matmul(ps_hpt[:], Hp[:], ident[:], start=True, stop=True)
                HpT = gp.tile([NN, NN], f32)
                nc.vector.tensor_copy(out=HpT[:], in_=ps_hpt[:])
                HpTs = gp.tile([NN, NN], f32)
                nc.scalar.mul(HpTs[:], ps_hpt[:], inv_de[:])  # HpT * inv_de[e]
                ps_G = gpsum.tile([NN, NN], f32)
                nc.tensor.matmul(ps_G[:], HpTs[:], HpT[:], start=True, stop=True)
                nc.vector.tensor_copy(out=G_h[:], in_=ps_G[:])
                G32 = gp.tile([NN, NN], f32)
                nc.scalar.copy(G32[:], ps_G[:])
                for s in GP_SHIFTS:
                    sel = gp.tile([NN, 128], f32, tag="sel")
                    nc.gpsimd.memset(sel[:], 0.0)
                    nc.gpsimd.affine_select(
                        out=sel[:], in_=sel[:],
                        compare_op=ALU.not_equal, fill=1.0,
                        base=s, pattern=[[-1, 128]], channel_multiplier=1)
                    ps_sel = gpsum.tile([128, NN], f32, tag="ps_sel")
                    nc.tensor.matmul(ps_sel[:], sel[:], G32[:], start=True, stop=True)
                    nc.vector.tensor_copy(out=gpad[s][:], in_=ps_sel[:])
                ps_g = gpsum.tile([NN, 1], f32)
                nc.tensor.matmul(ps_g[:], G32[:], ones_col[:], start=True, stop=True)
                g_col = gp.tile([NN, 1], f32)
                nc.vector.tensor_copy(out=g_col[:], in_=ps_g[:])
                ps_gr = gpsum.tile([1, NN], f32)
                nc.tensor.matmul(ps_gr[:], g_col[:], ident[:], start=True, stop=True)
                g_row = gp.tile([1, NN], f16)
                nc.vector.tensor_copy(out=g_row[:], in_=ps_gr[:])
                for b in range(B):
                    nc.vector.tensor_copy(out=grow_h[:, b * NN:(b + 1) * NN], in_=g_row[:])
                nc.vector.memset(ones128_h[:], 1.0)

            def build_stage(x_d, w1_d, b1_s, w2_d, b2_s, col_off, D, c0):
                """One 16-batch chunk of one stage. c0 = first batch index."""
                KT = D // 128
                DC = D // 512
                afm_cm = tc.tile_pool(name=f"afm{D}", bufs=1, side="right")
                afm_pool = afm_cm.__enter__()
                A_fm = afm_pool.tile([128, KT, R], f16)
                # phase A: AGG-B (RM -> FM)
                with tc.tile_pool(name=f"xp{D}", bufs=2) as xpool, \
                     tc.tile_pool(name=f"psA{D}", bufs=2, space="PSUM") as psumA:
                    for (b0, blen) in BGROUPS:
                        xg = xpool.tile([NN, 6, D], f16, tag="xg")
                        for j in range(blen):
                            nc.sync.dma_start(xg[:, j], x_d[c0 + b0 + j])
                        for kt in range(KT):
                            psA = psumA.tile([128, 6 * NN], f32)
                            for j in range(blen):
                                nc.tensor.matmul(
                                    psA[:, j * NN:(j + 1) * NN],
                                    xg[:, j, kt * 128:(kt + 1) * 128],
                                    G_h[:],
                                    start=True, stop=True)
                            nc.vector.tensor_copy(
                                out=A_fm[:, kt, b0 * NN:(b0 + blen) * NN],
                                in_=psA[:, :blen * NN])
                hfm_cm = tc.tile_pool(name=f"hfm{D}", bufs=1)
                hfm_pool = hfm_cm.__enter__()
                H_fm = hfm_pool.tile([128, KT, R], f16)
                # phase B: MUL-A + bias + relu (FM -> FM)
                with tc.tile_pool(name=f"wp{D}", bufs=2) as wpool, \
                     tc.tile_pool(name=f"psB{D}", bufs=4, space="PSUM") as psumB:
                    for dto in range(KT):
                        w1t = wpool.tile([128, KT, 128], f16, tag="w1t")
                        for kt in range(KT):
                            nc.sync.dma_start(
                                w1t[:, kt],
                                w1_d[kt * 128:(kt + 1) * 128,
                                     dto * 128:(dto + 1) * 128])
                        for (r0, rl) in RCHUNKS:
                            ps = psumB.tile([128, 512], f32)
                            for kt in range(KT):
                                nc.tensor.matmul(
                                    ps[:, :rl], w1t[:, kt],
                                    A_fm[:, kt, r0:r0 + rl],
                                    start=(kt == 0), stop=False)
                            nc.tensor.matmul(
                                ps[:, :rl],
                                b1_s[:, dto * 128:(dto + 1) * 128],
                                grow_h[:, r0:r0 + rl],
                                start=False, stop=True)
                            nc.scalar.activation(
                                H_fm[:, dto, r0:r0 + rl], ps[:, :rl], AF.Relu)
                afm_cm.__exit__(None, None, None)
                # phase C: MUL-B dense (M=128 r-rows), bias, AGG-A, DMA out.
                NT = R // 128  # 10
                with tc.tile_pool(name=f"w2p{D}", bufs=2) as w2pool, \
                     tc.tile_pool(name=f"yd{D}", bufs=NT + 1) as ydpool, \
                     tc.tile_pool(name=f"yz{D}", bufs=3) as yzpool, \
                     tc.tile_pool(name=f"psY{D}", bufs=2, space="PSUM") as psumY, \
                     tc.tile_pool(name=f"psZ{D}", bufs=2, space="PSUM") as psumZ:
                    for dc in range(DC):
                        w2c = w2pool.tile([128, KT, 512], f16, tag="w2c")
                        for kt in range(KT):
                            nc.sync.dma_start(
                                w2c[:, kt],
                                w2_d[kt * 128:(kt + 1) * 128,
                                     dc * 512:(dc + 1) * 512])
                        dense = []
                        for t in range(NT):
                            psy = psumY.tile([128, 512], f32)
                            for kt in range(KT):
                                nc.tensor.matmul(
                                    psy[:], H_fm[:, kt, t * 128:(t + 1) * 128],
                                    w2c[:, kt], start=(kt == 0), stop=False)
                            nc.tensor.matmul(
                                psy[:], ones128_h[:],
                                b2_s[:, dc * 512:(dc + 1) * 512],
                                start=False, stop=True)
                            ydn = ydpool.tile([128, 512], f16, tag="yd")
                            nc.vector.tensor_copy(out=ydn[:], in_=psy[:])
                            dense.append(ydn)
                        for b in range(B):
                            r0 = b * NN
                            t0, o0 = divmod(r0, 128)
                            psz = psumZ.tile([NN, 512], f32)
                            if o0 <= 48:
                                nc.tensor.matmul(psz[:], gpad[o0][:], dense[t0][:],
                                                 start=True, stop=True)
                            else:
                                nc.tensor.matmul(psz[:], gpad[o0][:], dense[t0][:],
                                                 start=True, stop=False)
                                nc.tensor.matmul(psz[:], gpad[o0 - 128][:],
                                                 dense[t0 + 1][:],
                                                 start=False, stop=True)
                            zsb = yzpool.tile([NN, 512], f16, tag="z")
                            nc.scalar.copy(zsb[:], psz[:])
                            nc.sync.dma_start(
                                out_d[c0 + b, :,
                                      col_off + dc * 512:col_off + (dc + 1) * 512],
                                zsb[:])
                hfm_cm.__exit__(None, None, None)

            for chunk in range(N_CHUNKS):
                c0 = chunk * B_CHUNK
                build_stage(x3_d, w31_d, b1s[1024], w32_d, b2s[1024], 0, 1024, c0)
                build_stage(x4_d, w41_d, b1s[2048], w42_d, b2s[2048], 1024, 2048, c0)

    nc.compile()
    return nc


def get_program():
    if "nc" not in _CACHE:
        _CACHE["nc"] = _build_program()
    return _CACHE["nc"]


def make_in_maps(inputs):
    x3 = np.asarray(inputs["stage_3_input"], dtype=np.float32).astype(np.float16)
    x4 = np.asarray(inputs["input_x"], dtype=np.float32).astype(np.float16)
    H = np.ascontiguousarray(np.asarray(inputs["H"], dtype=np.float32))
    m = {"x3": np.ascontiguousarray(x3), "x4": np.ascontiguousarray(x4), "H": H}
    for k in ("w31", "w32", "w41", "w42"):
        m[k] = np.ascontiguousarray(np.asarray(inputs[k]).astype(np.float16))
    for k in ("b31", "b32", "b41", "b42"):
        m[k] = np.ascontiguousarray(
            np.asarray(inputs[k]).astype(np.float16).reshape(1, -1))
    return [m]


def kernel(**inputs):
    from concourse.bass_utils import run_bass_kernel_spmd
    nc = get_program()
    in_maps = make_in_maps(inputs)
    res = run_bass_kernel_spmd(nc, in_maps, [0])
    out = res.results[0]["out"]
    return np.ascontiguousarray(out.astype(np.float32))
